# revision 17
# baseline (speedup 1.0000x reference)
"""Chamfer loss kernel for Trainium2 (8 NeuronCores).

loss = 0.5*(mean_i sqrt(min_j ||t_i-o_j||^2) + mean_j sqrt(min_i ||o_j-t_i||^2))
       * 10 / 1.02**(cur//20)

Strategy
--------
Both NN searches are sharded over the query-point dimension across the 8
cores.  Queries are Morton-ordered into 128-row tiles; for each tile the
host gathers the candidate points inside the tile's bounding box expanded
by R = max over the tile's rows of a rigorous per-row NN upper bound
(min of: distance to the generating partner point, and the best candidate
among +-128 Morton-rank neighbours, both computed exactly on host).
Every row's true NN provably lies in its tile's gathered set, so the
device window-min IS the global min -- no fallback needed.

The 512 tiles (2 directions x 256) are sorted by candidate count and
dealt in groups of 8 to the cores, so all cores execute the identical
static slot schedule (SPMD) and are load-balanced by construction.

On device, per tile: matmul with a K=18 bf16 hi/lo expansion of the
homogeneous distance form emits complete squared distances (negated) to
PSUM; the scalar engine drains PSUM to fp16 while the vector engine
max-folds; a ~1/8 fraction of chunks reduces directly from PSUM so both
engines stay busy.  Device outputs are per-row max(-d) = -min d.
"""

import numpy as np

N = 32768
NCORES = 8
RPC = N // NCORES          # query rows per core
TILES = RPC // 128         # tile slots per core per direction (32)
CHUNK = 2048               # PSUM chunk width (4 banks)
SENT = 100.0               # sentinel coordinate for slot padding
K = 18                     # contraction rows of the bf16 hi/lo expansion
UBWIN = 128                # half-window (in Morton ranks) for the ub bound

_cached = {}


# ----------------------------------------------------------------- device

def _build_program(widths1, widths2):
    import concourse.bacc as bacc
    import concourse.tile as tile
    from concourse import mybir

    f32 = mybir.dt.float32
    f16 = mybir.dt.float16
    bf16 = mybir.dt.bfloat16
    nc = bacc.Bacc("TRN2", target_bir_lowering=False, debug=False)

    tot1 = sum(widths1)
    tot2 = sum(widths2)
    lhs = [
        nc.dram_tensor(f"lhs{d}", (K, RPC), bf16, kind="ExternalInput")
        for d in (1, 2)
    ]
    cand = [
        nc.dram_tensor(f"cand{d}", (K, tot), bf16, kind="ExternalInput")
        for d, tot in ((1, tot1), (2, tot2))
    ]
    res = [
        nc.dram_tensor(f"res{d}", (128, TILES), f32, kind="ExternalOutput")
        for d in (1, 2)
    ]
    res_warm = nc.dram_tensor("res_warm", (128, 1), f32, kind="ExternalOutput")

    chunk_no = 0
    with tile.TileContext(nc) as tc:
        with (
            tc.tile_pool(name="lhs", bufs=1) as lhs_pool,
            tc.tile_pool(name="cand", bufs=3) as cand_pool,
            tc.tile_pool(name="acc", bufs=1) as acc_pool,
            tc.tile_pool(name="junk", bufs=2) as junk_pool,
            tc.tile_pool(name="stage", bufs=3) as stage_pool,
            tc.tile_pool(name="ps", bufs=2, space="PSUM") as ps_pool,
        ):
            for d in range(2):
                widths = (widths1, widths2)[d]
                lhs_sb = lhs_pool.tile([K, RPC], bf16, tag=f"lhs{d}")
                nc.sync.dma_start(out=lhs_sb, in_=lhs[d][:])
                racc = acc_pool.tile([128, TILES], f32, tag=f"racc{d}")

                if d == 0:
                    # HAM warm-up: ~24 back-to-back matmuls (~10us cold) push
                    # the PE clock gate from 4/8 (1.2 GHz) to 8/8 (2.4 GHz);
                    # it stays warm for the whole kernel (no >3.4us PE-idle
                    # window exists once the pipeline runs).
                    ps_w = ps_pool.tile([128, CHUNK], f32, tag="ps")
                    for _ in range(24):
                        nc.tensor.matmul(
                            ps_w[:, :512], lhs_sb[:, 0:128], lhs_sb[:, 0:512],
                            start=True, stop=True,
                        )
                    wtmp = junk_pool.tile([128, 1], f32, tag="wtmp")
                    nc.vector.tensor_reduce(
                        out=wtmp, in_=ps_w[:, :8],
                        axis=mybir.AxisListType.X, op=mybir.AluOpType.max,
                    )
                    nc.sync.dma_start(out=res_warm[:], in_=wtmp)

                off = 0
                for i, Ws in enumerate(widths):
                    lhsT = lhs_sb[:, i * 128:(i + 1) * 128]
                    nch = (Ws + CHUNK - 1) // CHUNK
                    rb = None
                    if nch > 1:
                        rb = junk_pool.tile([128, nch], f32, tag="rb", name="rb")
                    for ch in range(nch):
                        c = min(CHUNK, Ws - ch * CHUNK)
                        cnd = cand_pool.tile([K, CHUNK], bf16, tag="cnd")
                        nc.sync.dma_start(
                            out=cnd[:, :c],
                            in_=cand[d][:, off + ch * CHUNK: off + ch * CHUNK + c],
                        )
                        ps = ps_pool.tile([128, CHUNK], f32, tag="ps")
                        for j in range(c // 512):
                            nc.tensor.matmul(
                                ps[:, j * 512:(j + 1) * 512],
                                lhsT,
                                cnd[:, j * 512:(j + 1) * 512],
                                start=True,
                                stop=True,
                            )
                        dst = racc[:, i:i + 1] if nch == 1 else rb[:, ch:ch + 1]
                        # PSUM holds -d; max-reduce everywhere (host negates).
                        sel = chunk_no % 8
                        if sel == 7:
                            # direct: DVE reduces straight from PSUM (1x)
                            nc.vector.tensor_reduce(
                                out=dst, in_=ps[:, :c],
                                axis=mybir.AxisListType.X, op=mybir.AluOpType.max,
                            )
                        else:
                            # ACT drains PSUM to fp16, DVE max-folds at 2x
                            s = stage_pool.tile([128, CHUNK], f16, tag="s")
                            nc.scalar.copy(out=s[:, :c], in_=ps[:, :c])
                            h, q = c // 2, c // 4
                            f1 = stage_pool.tile([128, CHUNK // 2], f16, tag="f1")
                            nc.vector.tensor_max(f1[:, :h], s[:, :h], s[:, h:c])
                            f2 = stage_pool.tile([128, CHUNK // 4], f16, tag="f2")
                            nc.vector.tensor_max(f2[:, :q], f1[:, :q], f1[:, q:h])
                            nc.vector.tensor_reduce(
                                out=dst, in_=f2[:, :q],
                                axis=mybir.AxisListType.X, op=mybir.AluOpType.max,
                            )
                        chunk_no += 1
                    if nch > 1:
                        nc.vector.tensor_reduce(
                            out=racc[:, i:i + 1], in_=rb,
                            axis=mybir.AxisListType.X, op=mybir.AluOpType.max,
                        )
                    off += Ws
                nc.sync.dma_start(out=res[d][:], in_=racc)

    nc.compile()
    return nc


def _get_program(widths1, widths2):
    key = (widths1, widths2)
    if key not in _cached:
        _cached[key] = _build_program(widths1, widths2)
    return _cached[key]


# ------------------------------------------------------------------- host

def _bf16():
    import ml_dtypes
    return ml_dtypes.bfloat16


def _split2(v32):
    bf = _bf16()
    hi = v32.astype(bf)
    lo = (v32 - hi.astype(np.float32)).astype(bf)
    return hi, lo


def _split3(v64):
    bf = _bf16()
    a = v64.astype(np.float32).astype(bf)
    r = v64 - a.astype(np.float64)
    b = r.astype(np.float32).astype(bf)
    r = r - b.astype(np.float64)
    c = r.astype(np.float32).astype(bf)
    return a, b, c


def _pack(points):
    """[n,3] -> (lhs rows [K,n], cand rows [K,n]) in bf16 such that
    lhsT.T @ cand accumulates the squared distance d = |q|^2+|c|^2-2q.c
    to ~1e-7 via hi/lo splits.  Row pairing k: lhs[k]*cand[k]:
      0-2 qh*(-2ch)  3-5 ql*(-2ch)  6-8 qh*(-2cl)  9-11 ql*(-2cl)
      12-14 q2(3-way)*1   15-17 1*c2(3-way)
    """
    bf = _bf16()
    n = points.shape[0]
    xh, xl = _split2(points.T.astype(np.float32))
    q64 = xh.astype(np.float64) + xl.astype(np.float64)
    p2 = (q64 * q64).sum(0)
    p2a, p2b, p2c = _split3(p2)

    L = np.empty((K, n), bf)
    L[0:3] = xh
    L[3:6] = xl
    L[6:9] = xh
    L[9:12] = xl
    L[12] = p2a
    L[13] = p2b
    L[14] = p2c
    L[15:18] = np.ones((3, n), bf)

    R = np.empty((K, n), bf)
    m2h = (-2.0 * xh.astype(np.float32)).astype(bf)
    m2l = (-2.0 * xl.astype(np.float32)).astype(bf)
    R[0:3] = m2h
    R[3:6] = m2h
    R[6:9] = m2l
    R[9:12] = m2l
    R[12:15] = np.ones((3, n), bf)
    R[15] = p2a
    R[16] = p2b
    R[17] = p2c
    return L, R


def _morton(pts):
    q = np.clip((pts / 1.1 * 1024).astype(np.int64), 0, 1023)

    def spread(v):
        v = (v | (v << 16)) & 0x030000FF
        v = (v | (v << 8)) & 0x0300F00F
        v = (v | (v << 4)) & 0x030C30C3
        v = (v | (v << 2)) & 0x09249249
        return v

    return (spread(q[:, 0]) << 2) | (spread(q[:, 1]) << 1) | spread(q[:, 2])


def _ub_bound(rows, cands, pair_ub):
    """Rigorous per-row upper bound on the NN distance: min of the
    generating-pair distance and the exact best among +-UBWIN
    Morton-rank candidate neighbours (f32 eval, inflated for rounding)."""
    n = len(rows)
    co = np.argsort(_morton(cands), kind="stable")
    cs = cands[co].astype(np.float32)
    cms = _morton(cands)[co]
    pos = np.searchsorted(cms, _morton(rows))
    ub = np.empty(n, np.float64)
    win = np.arange(-UBWIN, UBWIN)
    rs32 = rows.astype(np.float32)
    for s in range(0, n, 2048):
        e = min(s + 2048, n)
        idx = np.clip(pos[s:e, None] + win[None, :], 0, n - 1)
        d = ((rs32[s:e, None, :] - cs[idx]) ** 2).sum(-1)
        ub[s:e] = d.min(1)
    ub = np.sqrt(ub) * 1.00001 + 1e-7          # cover f32 rounding
    return np.minimum(ub, pair_ub)


def _prep_direction(rows, cands, pair_ub):
    """Tile the queries (Morton), gather per-tile candidate boxes,
    deal tiles to cores.  Returns widths (per slot), per-core lhs/cand
    arrays and the row-index map."""
    bf = _bf16()
    ntile = N // 128
    order = np.argsort(_morton(rows), kind="stable")
    ubd = _ub_bound(rows, cands, pair_ub)

    rows64 = rows.astype(np.float64)
    cands64 = cands.astype(np.float64)
    tile_rows = order.reshape(ntile, 128)
    cand_idx = []
    w = np.empty(ntile, np.int64)
    for g in range(ntile):
        blk = rows64[tile_rows[g]]
        R = ubd[tile_rows[g]].max()
        lo = blk.min(0) - R
        hi = blk.max(0) + R
        m = ((cands64 >= lo) & (cands64 <= hi)).all(1)
        ci = np.flatnonzero(m)
        cand_idx.append(ci)
        w[g] = max(512, (len(ci) + 511) // 512 * 512)

    # deal: sort tiles by width desc; group i of 8 -> slot i on each core
    tord = np.argsort(-w, kind="stable")
    widths = tuple(int(w[tord[i * NCORES]]) for i in range(TILES))
    tot = sum(widths)

    L, _ = _pack(rows)
    L = (-L.astype(np.float32)).astype(bf)     # PE emits -d
    _, R = _pack(cands)
    _, sentR = _pack(np.full((1, 3), SENT, np.float32))

    lhs_maps, cand_maps, row_maps = [], [], []
    for c in range(NCORES):
        lhs_m = np.empty((K, RPC), bf)
        cand_m = np.empty((K, tot), bf)
        cand_m[:] = sentR
        rmap = np.empty((TILES, 128), np.int64)
        off = 0
        for i in range(TILES):
            g = tord[i * NCORES + c]
            lhs_m[:, i * 128:(i + 1) * 128] = L[:, tile_rows[g]]
            ci = cand_idx[g]
            cand_m[:, off:off + len(ci)] = R[:, ci]
            rmap[i] = tile_rows[g]
            off += widths[i]
        lhs_maps.append(lhs_m)
        cand_maps.append(cand_m)
        row_maps.append(rmap)
    return widths, lhs_maps, cand_maps, row_maps


def _install_ntff_hook_shim():
    """The agent image's `antenv` lacks `axon_hooks`, which bass_utils
    imports unconditionally when trace=True under axon.  Provide it,
    wired to the ctypes NTFF profiler from trn_agent_boot."""
    import sys, types
    if "antenv.axon_hooks" in sys.modules:
        return
    hook = None
    try:
        from trn_agent_boot.trn_boot import _ntff_profile_via_ctypes
        hook = _ntff_profile_via_ctypes("/opt/axon/libaxon_pjrt.so")
    except Exception:
        pass
    mod = types.ModuleType("antenv.axon_hooks")
    mod._hook = hook
    mod.get_axon_ntff_profile_hook = lambda: mod._hook

    def set_axon_ntff_profile_hook(h):
        mod._hook = h

    mod.set_axon_ntff_profile_hook = set_axon_ntff_profile_hook
    sys.modules["antenv.axon_hooks"] = mod
    try:
        import antenv
        antenv.axon_hooks = mod
    except Exception:
        pass


def _run(target, output, cur, trace=False):
    if trace:
        _install_ntff_hook_shim()
    from concourse.bass_utils import run_bass_kernel_spmd

    target = np.asarray(target, np.float32)
    output = np.asarray(output, np.float32)
    pair_ub = np.sqrt(
        ((target.astype(np.float64) - output.astype(np.float64)) ** 2).sum(-1)
    ) * 1.0000001

    w1, lhs1, cnd1, rmap1 = _prep_direction(target, output, pair_ub)
    w2, lhs2, cnd2, rmap2 = _prep_direction(output, target, pair_ub)

    in_maps = [
        {"lhs1": lhs1[c], "cand1": cnd1[c], "lhs2": lhs2[c], "cand2": cnd2[c]}
        for c in range(NCORES)
    ]
    nc = _get_program(w1, w2)
    r = run_bass_kernel_spmd(nc, in_maps, core_ids=list(range(NCORES)),
                             trace=trace)

    def collect(key, rmaps):
        out = np.empty(N, np.float64)
        for c in range(NCORES):
            blk = np.asarray(r.results[c][key], np.float64)   # [128, TILES]
            out[rmaps[c].reshape(-1)] = -blk.T.reshape(-1)
        return np.maximum(out, 0.0)

    m1 = collect("res1", rmap1)
    m2 = collect("res2", rmap2)
    loss = 0.5 * (np.sqrt(m1).mean() + np.sqrt(m2).mean())
    loss = loss * 10.0 / (1.02 ** (int(cur) // 20))
    return np.float32(loss), r


def kernel(target, output, cur):
    out, _ = _run(target, output, cur)
    return out


# revision 22
# speedup vs baseline: 1.1156x; 1.1156x over previous
"""Chamfer loss kernel for Trainium2 (8 NeuronCores).

loss = 0.5*(mean_i sqrt(min_j ||t_i-o_j||^2) + mean_j sqrt(min_i ||o_j-t_i||^2))
       * 10 / 1.02**(cur//20)

Strategy
--------
Both NN searches are sharded over the query-point dimension across the 8
cores.  Queries are Morton-ordered into 128-row tiles; for each tile the
host gathers the candidate points inside the tile's bounding box expanded
by R = max over the tile's rows of a rigorous per-row NN upper bound
(min of: distance to the generating partner point, and the best candidate
among +-128 Morton-rank neighbours, both computed exactly on host).
Every row's true NN provably lies in its tile's gathered set, so the
device window-min IS the global min -- no fallback needed.

The 512 tiles (2 directions x 256) are sorted by candidate count and
dealt in groups of 8 to the cores, so all cores execute the identical
static slot schedule (SPMD) and are load-balanced by construction.

On device, per tile: matmul with a K=18 bf16 hi/lo expansion of the
homogeneous distance form emits complete squared distances (negated) to
PSUM; the scalar engine drains PSUM to fp16 while the vector engine
max-folds; a ~1/8 fraction of chunks reduces directly from PSUM so both
engines stay busy.  Device outputs are per-row max(-d) = -min d.
"""

import numpy as np

N = 32768
NCORES = 8
RPC = N // NCORES          # query rows per core
TILES = RPC // 128         # tile slots per core per direction (32)
CHUNK = 2048               # PSUM chunk width (4 banks)
SENT = 100.0               # sentinel coordinate for slot padding
K = 18                     # contraction rows of the bf16 hi/lo expansion
UBWIN = 256                # half-window (in Morton ranks) for the ub bound

_cached = {}


# ----------------------------------------------------------------- device

def _build_program(widths1, widths2):
    import concourse.bacc as bacc
    import concourse.tile as tile
    from concourse import mybir

    f32 = mybir.dt.float32
    f16 = mybir.dt.float16
    bf16 = mybir.dt.bfloat16
    nc = bacc.Bacc("TRN2", target_bir_lowering=False, debug=False)

    tot1 = sum(widths1)
    tot2 = sum(widths2)
    lhs = [
        nc.dram_tensor(f"lhs{d}", (K, RPC), bf16, kind="ExternalInput")
        for d in (1, 2)
    ]
    cand = [
        nc.dram_tensor(f"cand{d}", (K, tot), bf16, kind="ExternalInput")
        for d, tot in ((1, tot1), (2, tot2))
    ]
    res = [
        nc.dram_tensor(f"res{d}", (128, TILES), f32, kind="ExternalOutput")
        for d in (1, 2)
    ]
    chunk_no = 0
    with tile.TileContext(nc) as tc:
        with (
            tc.tile_pool(name="lhs", bufs=1) as lhs_pool,
            tc.tile_pool(name="cand", bufs=3) as cand_pool,
            tc.tile_pool(name="acc", bufs=1) as acc_pool,
            tc.tile_pool(name="junk", bufs=2) as junk_pool,
            tc.tile_pool(name="stage", bufs=3) as stage_pool,
            tc.tile_pool(name="ps", bufs=2, space="PSUM") as ps_pool,
        ):
            for d in range(2):
                widths = (widths1, widths2)[d]
                lhs_sb = lhs_pool.tile([K, RPC], bf16, tag=f"lhs{d}")
                nc.sync.dma_start(out=lhs_sb, in_=lhs[d][:])
                racc = acc_pool.tile([128, TILES], f32, tag=f"racc{d}")

                off = 0
                for i, Ws in enumerate(widths):
                    lhsT = lhs_sb[:, i * 128:(i + 1) * 128]
                    nch = (Ws + CHUNK - 1) // CHUNK
                    rb = None
                    if nch > 1:
                        rb = junk_pool.tile([128, nch], f32, tag="rb", name="rb")
                    for ch in range(nch):
                        c = min(CHUNK, Ws - ch * CHUNK)
                        cnd = cand_pool.tile([K, CHUNK], bf16, tag="cnd")
                        nc.sync.dma_start(
                            out=cnd[:, :c],
                            in_=cand[d][:, off + ch * CHUNK: off + ch * CHUNK + c],
                        )
                        ps = ps_pool.tile([128, CHUNK], f32, tag="ps")
                        for j0 in range(0, c, 512):
                            n = min(512, c - j0)
                            nc.tensor.matmul(
                                ps[:, j0:j0 + n],
                                lhsT,
                                cnd[:, j0:j0 + n],
                                start=True,
                                stop=True,
                            )
                        dst = racc[:, i:i + 1] if nch == 1 else rb[:, ch:ch + 1]
                        # PSUM holds -d; max-reduce everywhere (host negates).
                        sel = chunk_no % 8
                        if sel == 7:
                            # direct: DVE reduces straight from PSUM (1x)
                            nc.vector.tensor_reduce(
                                out=dst, in_=ps[:, :c],
                                axis=mybir.AxisListType.X, op=mybir.AluOpType.max,
                            )
                        else:
                            # ACT drains PSUM to fp16, DVE max-folds at 2x
                            s = stage_pool.tile([128, CHUNK], f16, tag="s")
                            nc.scalar.copy(out=s[:, :c], in_=ps[:, :c])
                            h, q = c // 2, c // 4
                            f1 = stage_pool.tile([128, CHUNK // 2], f16, tag="f1")
                            nc.vector.tensor_max(f1[:, :h], s[:, :h], s[:, h:c])
                            f2 = stage_pool.tile([128, CHUNK // 4], f16, tag="f2")
                            nc.vector.tensor_max(f2[:, :q], f1[:, :q], f1[:, q:h])
                            nc.vector.tensor_reduce(
                                out=dst, in_=f2[:, :q],
                                axis=mybir.AxisListType.X, op=mybir.AluOpType.max,
                            )
                        chunk_no += 1
                    if nch > 1:
                        nc.vector.tensor_reduce(
                            out=racc[:, i:i + 1], in_=rb,
                            axis=mybir.AxisListType.X, op=mybir.AluOpType.max,
                        )
                    off += Ws
                nc.sync.dma_start(out=res[d][:], in_=racc)

    nc.compile()
    return nc


def _get_program(widths1, widths2):
    key = (widths1, widths2)
    if key not in _cached:
        _cached[key] = _build_program(widths1, widths2)
    return _cached[key]


# ------------------------------------------------------------------- host

def _bf16():
    import ml_dtypes
    return ml_dtypes.bfloat16


def _split2(v32):
    bf = _bf16()
    hi = v32.astype(bf)
    lo = (v32 - hi.astype(np.float32)).astype(bf)
    return hi, lo


def _split3(v64):
    bf = _bf16()
    a = v64.astype(np.float32).astype(bf)
    r = v64 - a.astype(np.float64)
    b = r.astype(np.float32).astype(bf)
    r = r - b.astype(np.float64)
    c = r.astype(np.float32).astype(bf)
    return a, b, c


def _pack(points):
    """[n,3] -> (lhs rows [K,n], cand rows [K,n]) in bf16 such that
    lhsT.T @ cand accumulates the squared distance d = |q|^2+|c|^2-2q.c
    to ~1e-7 via hi/lo splits.  Row pairing k: lhs[k]*cand[k]:
      0-2 qh*(-2ch)  3-5 ql*(-2ch)  6-8 qh*(-2cl)  9-11 ql*(-2cl)
      12-14 q2(3-way)*1   15-17 1*c2(3-way)
    """
    bf = _bf16()
    n = points.shape[0]
    xh, xl = _split2(points.T.astype(np.float32))
    q64 = xh.astype(np.float64) + xl.astype(np.float64)
    p2 = (q64 * q64).sum(0)
    p2a, p2b, p2c = _split3(p2)

    L = np.empty((K, n), bf)
    L[0:3] = xh
    L[3:6] = xl
    L[6:9] = xh
    L[9:12] = xl
    L[12] = p2a
    L[13] = p2b
    L[14] = p2c
    L[15:18] = np.ones((3, n), bf)

    R = np.empty((K, n), bf)
    m2h = (-2.0 * xh.astype(np.float32)).astype(bf)
    m2l = (-2.0 * xl.astype(np.float32)).astype(bf)
    R[0:3] = m2h
    R[3:6] = m2h
    R[6:9] = m2l
    R[9:12] = m2l
    R[12:15] = np.ones((3, n), bf)
    R[15] = p2a
    R[16] = p2b
    R[17] = p2c
    return L, R


def _morton(pts):
    q = np.clip((pts / 1.1 * 1024).astype(np.int64), 0, 1023)

    def spread(v):
        v = (v | (v << 16)) & 0x030000FF
        v = (v | (v << 8)) & 0x0300F00F
        v = (v | (v << 4)) & 0x030C30C3
        v = (v | (v << 2)) & 0x09249249
        return v

    return (spread(q[:, 0]) << 2) | (spread(q[:, 1]) << 1) | spread(q[:, 2])


def _ub_bound(rows, cands, pair_ub):
    """Rigorous per-row upper bound on the NN distance: min of the
    generating-pair distance and the exact best among +-UBWIN
    Morton-rank candidate neighbours (f32 eval, inflated for rounding)."""
    n = len(rows)
    co = np.argsort(_morton(cands), kind="stable")
    cs = cands[co].astype(np.float32)
    cms = _morton(cands)[co]
    pos = np.searchsorted(cms, _morton(rows))
    ub = np.empty(n, np.float64)
    win = np.arange(-UBWIN, UBWIN)
    rs32 = rows.astype(np.float32)
    for s in range(0, n, 2048):
        e = min(s + 2048, n)
        idx = np.clip(pos[s:e, None] + win[None, :], 0, n - 1)
        d = ((rs32[s:e, None, :] - cs[idx]) ** 2).sum(-1)
        ub[s:e] = d.min(1)
    ub = np.sqrt(ub) * 1.00001 + 1e-7          # cover f32 rounding
    return np.minimum(ub, pair_ub)


def _prep_direction(rows, cands, pair_ub):
    """Tile the queries (Morton), gather per-tile candidate boxes,
    deal tiles to cores.  Returns widths (per slot), per-core lhs/cand
    arrays and the row-index map."""
    bf = _bf16()
    ntile = N // 128
    order = np.argsort(_morton(rows), kind="stable")
    ubd = _ub_bound(rows, cands, pair_ub)

    rows64 = rows.astype(np.float64)
    cands64 = cands.astype(np.float64)
    tile_rows = order.reshape(ntile, 128)
    cand_idx = []
    w = np.empty(ntile, np.int64)
    for g in range(ntile):
        blk = rows64[tile_rows[g]]
        R = ubd[tile_rows[g]].max()
        lo = blk.min(0) - R
        hi = blk.max(0) + R
        m = ((cands64 >= lo) & (cands64 <= hi)).all(1)
        ci = np.flatnonzero(m)
        cand_idx.append(ci)
        w[g] = max(256, (len(ci) + 255) // 256 * 256)

    # deal: sort tiles by width desc; group i of 8 -> slot i on each core
    tord = np.argsort(-w, kind="stable")
    widths = tuple(int(w[tord[i * NCORES]]) for i in range(TILES))
    tot = sum(widths)

    L, _ = _pack(rows)
    L = (-L.astype(np.float32)).astype(bf)     # PE emits -d
    _, R = _pack(cands)
    _, sentR = _pack(np.full((1, 3), SENT, np.float32))

    lhs_maps, cand_maps, row_maps = [], [], []
    for c in range(NCORES):
        lhs_m = np.empty((K, RPC), bf)
        cand_m = np.empty((K, tot), bf)
        cand_m[:] = sentR
        rmap = np.empty((TILES, 128), np.int64)
        off = 0
        for i in range(TILES):
            g = tord[i * NCORES + c]
            lhs_m[:, i * 128:(i + 1) * 128] = L[:, tile_rows[g]]
            ci = cand_idx[g]
            cand_m[:, off:off + len(ci)] = R[:, ci]
            rmap[i] = tile_rows[g]
            off += widths[i]
        lhs_maps.append(lhs_m)
        cand_maps.append(cand_m)
        row_maps.append(rmap)
    return widths, lhs_maps, cand_maps, row_maps


def _install_ntff_hook_shim():
    """The agent image's `antenv` lacks `axon_hooks`, which bass_utils
    imports unconditionally when trace=True under axon.  Provide it,
    wired to the ctypes NTFF profiler from trn_agent_boot."""
    import sys, types
    if "antenv.axon_hooks" in sys.modules:
        return
    hook = None
    try:
        from trn_agent_boot.trn_boot import _ntff_profile_via_ctypes
        hook = _ntff_profile_via_ctypes("/opt/axon/libaxon_pjrt.so")
    except Exception:
        pass
    mod = types.ModuleType("antenv.axon_hooks")
    mod._hook = hook
    mod.get_axon_ntff_profile_hook = lambda: mod._hook

    def set_axon_ntff_profile_hook(h):
        mod._hook = h

    mod.set_axon_ntff_profile_hook = set_axon_ntff_profile_hook
    sys.modules["antenv.axon_hooks"] = mod
    try:
        import antenv
        antenv.axon_hooks = mod
    except Exception:
        pass


def _run(target, output, cur, trace=False):
    if trace:
        _install_ntff_hook_shim()
    from concourse.bass_utils import run_bass_kernel_spmd

    target = np.asarray(target, np.float32)
    output = np.asarray(output, np.float32)
    pair_ub = np.sqrt(
        ((target.astype(np.float64) - output.astype(np.float64)) ** 2).sum(-1)
    ) * 1.0000001

    w1, lhs1, cnd1, rmap1 = _prep_direction(target, output, pair_ub)
    w2, lhs2, cnd2, rmap2 = _prep_direction(output, target, pair_ub)

    in_maps = [
        {"lhs1": lhs1[c], "cand1": cnd1[c], "lhs2": lhs2[c], "cand2": cnd2[c]}
        for c in range(NCORES)
    ]
    nc = _get_program(w1, w2)
    r = run_bass_kernel_spmd(nc, in_maps, core_ids=list(range(NCORES)),
                             trace=trace)

    def collect(key, rmaps):
        out = np.empty(N, np.float64)
        for c in range(NCORES):
            blk = np.asarray(r.results[c][key], np.float64)   # [128, TILES]
            out[rmaps[c].reshape(-1)] = -blk.T.reshape(-1)
        return np.maximum(out, 0.0)

    m1 = collect("res1", rmap1)
    m2 = collect("res2", rmap2)
    loss = 0.5 * (np.sqrt(m1).mean() + np.sqrt(m2).mean())
    loss = loss * 10.0 / (1.02 ** (int(cur) // 20))
    return np.float32(loss), r


def kernel(target, output, cur):
    out, _ = _run(target, output, cur)
    return out


# revision 23
# speedup vs baseline: 1.1282x; 1.0113x over previous
"""Chamfer loss kernel for Trainium2 (8 NeuronCores).

loss = 0.5*(mean_i sqrt(min_j ||t_i-o_j||^2) + mean_j sqrt(min_i ||o_j-t_i||^2))
       * 10 / 1.02**(cur//20)

Strategy
--------
Both NN searches are sharded over the query-point dimension across the 8
cores.  Queries are Morton-ordered into 128-row tiles; for each tile the
host gathers the candidate points inside the tile's bounding box expanded
by R = max over the tile's rows of a rigorous per-row NN upper bound
(min of: distance to the generating partner point, and the best candidate
among +-128 Morton-rank neighbours, both computed exactly on host).
Every row's true NN provably lies in its tile's gathered set, so the
device window-min IS the global min -- no fallback needed.

The 512 tiles (2 directions x 256) are sorted by candidate count and
dealt in groups of 8 to the cores, so all cores execute the identical
static slot schedule (SPMD) and are load-balanced by construction.

On device, per tile: matmul with a K=18 bf16 hi/lo expansion of the
homogeneous distance form emits complete squared distances (negated) to
PSUM; the scalar engine drains PSUM to fp16 while the vector engine
max-folds; a ~1/8 fraction of chunks reduces directly from PSUM so both
engines stay busy.  Device outputs are per-row max(-d) = -min d.
"""

import numpy as np

N = 32768
NCORES = 8
RPC = N // NCORES          # query rows per core
TILES = RPC // 128         # tile slots per core per direction (32)
CHUNK = 2048               # PSUM chunk width (4 banks)
SENT = 100.0               # sentinel coordinate for slot padding
K = 18                     # contraction rows of the bf16 hi/lo expansion
UBWIN = 256                # half-window (in Morton ranks) for the ub bound

_cached = {}


# ----------------------------------------------------------------- device

def _build_program(widths1, widths2):
    import concourse.bacc as bacc
    import concourse.tile as tile
    from concourse import mybir

    f32 = mybir.dt.float32
    f16 = mybir.dt.float16
    bf16 = mybir.dt.bfloat16
    nc = bacc.Bacc("TRN2", target_bir_lowering=False, debug=False)

    tot1 = sum(widths1)
    tot2 = sum(widths2)
    lhs = [
        nc.dram_tensor(f"lhs{d}", (K, RPC), bf16, kind="ExternalInput")
        for d in (1, 2)
    ]
    cand = [
        nc.dram_tensor(f"cand{d}", (K, tot), bf16, kind="ExternalInput")
        for d, tot in ((1, tot1), (2, tot2))
    ]
    res = [
        nc.dram_tensor(f"res{d}", (128, TILES), f32, kind="ExternalOutput")
        for d in (1, 2)
    ]
    chunk_no = 0
    with tile.TileContext(nc) as tc:
        with (
            tc.tile_pool(name="lhs", bufs=1) as lhs_pool,
            tc.tile_pool(name="cand", bufs=3) as cand_pool,
            tc.tile_pool(name="acc", bufs=1) as acc_pool,
            tc.tile_pool(name="junk", bufs=2) as junk_pool,
            tc.tile_pool(name="stage", bufs=3) as stage_pool,
            tc.tile_pool(name="ps", bufs=2, space="PSUM") as ps_pool,
        ):
            for d in range(2):
                widths = (widths1, widths2)[d]
                lhs_sb = lhs_pool.tile([K, RPC], bf16, tag=f"lhs{d}")
                nc.sync.dma_start(out=lhs_sb, in_=lhs[d][:])
                racc = acc_pool.tile([128, TILES], f32, tag=f"racc{d}")

                off = 0
                for i, Ws in enumerate(widths):
                    lhsT = lhs_sb[:, i * 128:(i + 1) * 128]
                    nch = (Ws + CHUNK - 1) // CHUNK
                    rb = None
                    if nch > 1:
                        rb = junk_pool.tile([128, nch], f32, tag="rb", name="rb")
                    for ch in range(nch):
                        c = min(CHUNK, Ws - ch * CHUNK)
                        cnd = cand_pool.tile([K, CHUNK], bf16, tag="cnd")
                        nc.sync.dma_start(
                            out=cnd[:, :c],
                            in_=cand[d][:, off + ch * CHUNK: off + ch * CHUNK + c],
                        )
                        ps = ps_pool.tile([128, CHUNK], f32, tag="ps")
                        for j0 in range(0, c, 512):
                            n = min(512, c - j0)
                            nc.tensor.matmul(
                                ps[:, j0:j0 + n],
                                lhsT,
                                cnd[:, j0:j0 + n],
                                start=True,
                                stop=True,
                            )
                        dst = racc[:, i:i + 1] if nch == 1 else rb[:, ch:ch + 1]
                        # PSUM holds -d; max-reduce everywhere (host negates).
                        sel = chunk_no % 4
                        if sel == 3:
                            # direct: DVE reduces straight from PSUM (1x)
                            nc.vector.tensor_reduce(
                                out=dst, in_=ps[:, :c],
                                axis=mybir.AxisListType.X, op=mybir.AluOpType.max,
                            )
                        else:
                            # ACT drains PSUM to fp16, DVE max-folds at 2x
                            s = stage_pool.tile([128, CHUNK], f16, tag="s")
                            nc.scalar.copy(out=s[:, :c], in_=ps[:, :c])
                            h, q = c // 2, c // 4
                            f1 = stage_pool.tile([128, CHUNK // 2], f16, tag="f1")
                            nc.vector.tensor_max(f1[:, :h], s[:, :h], s[:, h:c])
                            f2 = stage_pool.tile([128, CHUNK // 4], f16, tag="f2")
                            nc.vector.tensor_max(f2[:, :q], f1[:, :q], f1[:, q:h])
                            nc.vector.tensor_reduce(
                                out=dst, in_=f2[:, :q],
                                axis=mybir.AxisListType.X, op=mybir.AluOpType.max,
                            )
                        chunk_no += 1
                    if nch > 1:
                        nc.vector.tensor_reduce(
                            out=racc[:, i:i + 1], in_=rb,
                            axis=mybir.AxisListType.X, op=mybir.AluOpType.max,
                        )
                    off += Ws
                nc.sync.dma_start(out=res[d][:], in_=racc)

    nc.compile()
    return nc


def _get_program(widths1, widths2):
    key = (widths1, widths2)
    if key not in _cached:
        _cached[key] = _build_program(widths1, widths2)
    return _cached[key]


# ------------------------------------------------------------------- host

def _bf16():
    import ml_dtypes
    return ml_dtypes.bfloat16


def _split2(v32):
    bf = _bf16()
    hi = v32.astype(bf)
    lo = (v32 - hi.astype(np.float32)).astype(bf)
    return hi, lo


def _split3(v64):
    bf = _bf16()
    a = v64.astype(np.float32).astype(bf)
    r = v64 - a.astype(np.float64)
    b = r.astype(np.float32).astype(bf)
    r = r - b.astype(np.float64)
    c = r.astype(np.float32).astype(bf)
    return a, b, c


def _pack(points):
    """[n,3] -> (lhs rows [K,n], cand rows [K,n]) in bf16 such that
    lhsT.T @ cand accumulates the squared distance d = |q|^2+|c|^2-2q.c
    to ~1e-7 via hi/lo splits.  Row pairing k: lhs[k]*cand[k]:
      0-2 qh*(-2ch)  3-5 ql*(-2ch)  6-8 qh*(-2cl)  9-11 ql*(-2cl)
      12-14 q2(3-way)*1   15-17 1*c2(3-way)
    """
    bf = _bf16()
    n = points.shape[0]
    xh, xl = _split2(points.T.astype(np.float32))
    q64 = xh.astype(np.float64) + xl.astype(np.float64)
    p2 = (q64 * q64).sum(0)
    p2a, p2b, p2c = _split3(p2)

    L = np.empty((K, n), bf)
    L[0:3] = xh
    L[3:6] = xl
    L[6:9] = xh
    L[9:12] = xl
    L[12] = p2a
    L[13] = p2b
    L[14] = p2c
    L[15:18] = np.ones((3, n), bf)

    R = np.empty((K, n), bf)
    m2h = (-2.0 * xh.astype(np.float32)).astype(bf)
    m2l = (-2.0 * xl.astype(np.float32)).astype(bf)
    R[0:3] = m2h
    R[3:6] = m2h
    R[6:9] = m2l
    R[9:12] = m2l
    R[12:15] = np.ones((3, n), bf)
    R[15] = p2a
    R[16] = p2b
    R[17] = p2c
    return L, R


def _morton(pts):
    q = np.clip((pts / 1.1 * 1024).astype(np.int64), 0, 1023)

    def spread(v):
        v = (v | (v << 16)) & 0x030000FF
        v = (v | (v << 8)) & 0x0300F00F
        v = (v | (v << 4)) & 0x030C30C3
        v = (v | (v << 2)) & 0x09249249
        return v

    return (spread(q[:, 0]) << 2) | (spread(q[:, 1]) << 1) | spread(q[:, 2])


def _ub_bound(rows, cands, pair_ub):
    """Rigorous per-row upper bound on the NN distance: min of the
    generating-pair distance and the exact best among +-UBWIN
    Morton-rank candidate neighbours (f32 eval, inflated for rounding)."""
    n = len(rows)
    co = np.argsort(_morton(cands), kind="stable")
    cs = cands[co].astype(np.float32)
    cms = _morton(cands)[co]
    pos = np.searchsorted(cms, _morton(rows))
    ub = np.empty(n, np.float64)
    win = np.arange(-UBWIN, UBWIN)
    rs32 = rows.astype(np.float32)
    for s in range(0, n, 2048):
        e = min(s + 2048, n)
        idx = np.clip(pos[s:e, None] + win[None, :], 0, n - 1)
        d = ((rs32[s:e, None, :] - cs[idx]) ** 2).sum(-1)
        ub[s:e] = d.min(1)
    ub = np.sqrt(ub) * 1.00001 + 1e-7          # cover f32 rounding
    return np.minimum(ub, pair_ub)


def _prep_direction(rows, cands, pair_ub):
    """Tile the queries (Morton), gather per-tile candidate boxes,
    deal tiles to cores.  Returns widths (per slot), per-core lhs/cand
    arrays and the row-index map."""
    bf = _bf16()
    ntile = N // 128
    order = np.argsort(_morton(rows), kind="stable")
    ubd = _ub_bound(rows, cands, pair_ub)

    rows64 = rows.astype(np.float64)
    cands64 = cands.astype(np.float64)
    tile_rows = order.reshape(ntile, 128)
    cand_idx = []
    w = np.empty(ntile, np.int64)
    for g in range(ntile):
        blk = rows64[tile_rows[g]]
        R = ubd[tile_rows[g]].max()
        lo = blk.min(0) - R
        hi = blk.max(0) + R
        m = ((cands64 >= lo) & (cands64 <= hi)).all(1)
        ci = np.flatnonzero(m)
        cand_idx.append(ci)
        w[g] = max(256, (len(ci) + 255) // 256 * 256)

    # deal: sort tiles by width desc; group i of 8 -> slot i on each core
    tord = np.argsort(-w, kind="stable")
    widths = tuple(int(w[tord[i * NCORES]]) for i in range(TILES))
    tot = sum(widths)

    L, _ = _pack(rows)
    L = (-L.astype(np.float32)).astype(bf)     # PE emits -d
    _, R = _pack(cands)
    _, sentR = _pack(np.full((1, 3), SENT, np.float32))

    lhs_maps, cand_maps, row_maps = [], [], []
    for c in range(NCORES):
        lhs_m = np.empty((K, RPC), bf)
        cand_m = np.empty((K, tot), bf)
        cand_m[:] = sentR
        rmap = np.empty((TILES, 128), np.int64)
        off = 0
        for i in range(TILES):
            g = tord[i * NCORES + c]
            lhs_m[:, i * 128:(i + 1) * 128] = L[:, tile_rows[g]]
            ci = cand_idx[g]
            cand_m[:, off:off + len(ci)] = R[:, ci]
            rmap[i] = tile_rows[g]
            off += widths[i]
        lhs_maps.append(lhs_m)
        cand_maps.append(cand_m)
        row_maps.append(rmap)
    return widths, lhs_maps, cand_maps, row_maps


def _install_ntff_hook_shim():
    """The agent image's `antenv` lacks `axon_hooks`, which bass_utils
    imports unconditionally when trace=True under axon.  Provide it,
    wired to the ctypes NTFF profiler from trn_agent_boot."""
    import sys, types
    if "antenv.axon_hooks" in sys.modules:
        return
    hook = None
    try:
        from trn_agent_boot.trn_boot import _ntff_profile_via_ctypes
        hook = _ntff_profile_via_ctypes("/opt/axon/libaxon_pjrt.so")
    except Exception:
        pass
    mod = types.ModuleType("antenv.axon_hooks")
    mod._hook = hook
    mod.get_axon_ntff_profile_hook = lambda: mod._hook

    def set_axon_ntff_profile_hook(h):
        mod._hook = h

    mod.set_axon_ntff_profile_hook = set_axon_ntff_profile_hook
    sys.modules["antenv.axon_hooks"] = mod
    try:
        import antenv
        antenv.axon_hooks = mod
    except Exception:
        pass


def _run(target, output, cur, trace=False):
    if trace:
        _install_ntff_hook_shim()
    from concourse.bass_utils import run_bass_kernel_spmd

    target = np.asarray(target, np.float32)
    output = np.asarray(output, np.float32)
    pair_ub = np.sqrt(
        ((target.astype(np.float64) - output.astype(np.float64)) ** 2).sum(-1)
    ) * 1.0000001

    w1, lhs1, cnd1, rmap1 = _prep_direction(target, output, pair_ub)
    w2, lhs2, cnd2, rmap2 = _prep_direction(output, target, pair_ub)

    in_maps = [
        {"lhs1": lhs1[c], "cand1": cnd1[c], "lhs2": lhs2[c], "cand2": cnd2[c]}
        for c in range(NCORES)
    ]
    nc = _get_program(w1, w2)
    r = run_bass_kernel_spmd(nc, in_maps, core_ids=list(range(NCORES)),
                             trace=trace)

    def collect(key, rmaps):
        out = np.empty(N, np.float64)
        for c in range(NCORES):
            blk = np.asarray(r.results[c][key], np.float64)   # [128, TILES]
            out[rmaps[c].reshape(-1)] = -blk.T.reshape(-1)
        return np.maximum(out, 0.0)

    m1 = collect("res1", rmap1)
    m2 = collect("res2", rmap2)
    loss = 0.5 * (np.sqrt(m1).mean() + np.sqrt(m2).mean())
    loss = loss * 10.0 / (1.02 ** (int(cur) // 20))
    return np.float32(loss), r


def kernel(target, output, cur):
    out, _ = _run(target, output, cur)
    return out
